# revision 1
# baseline (speedup 1.0000x reference)
"""ComplexMDTA Trainium2 kernel.

Sharding: 8 cores = (batch 4) x (H halves 2). Each core computes its
(batch, 96-row) slice end-to-end. The only cross-core data dependencies
are the L2-norm sums and q@k^T Gram matrices (reductions over the full
H*W axis), handled with a tiny pairwise AllReduce between the two cores
sharing a batch.

Per core:
  phase 1: combine qkv 1x1-conv weights with depthwise 3x3 taps into 9
           fused complex tap matrices M[tap] (on device, DVE).
  phase 2: streamed over 16-row blocks: fused conv1x1+dwconv3x3 via 27
           PSUM-accumulated fp32r matmuls per 2-row chunk (Gauss 3-mult
           complex trick), epilogue to q/k/v tiles, PE transposes +
           Gram-matmul accumulation for q@kT, ACT-square sumsq.
  phase 3: pairwise AllReduce of Gram+sumsq partials, normalization,
           temperature, masked per-head softmax -> block-diag attn.
  phase 4: streamed attn@v + projection 1x1 conv, DMA out.
"""
import os
import sys

for _p in ('/opt/trn_rl_repo', '/root/.axon_site/_ro/trn_rl_repo'):
    if os.path.isdir(_p) and _p not in sys.path:
        sys.path.insert(0, _p)

import numpy as np
import concourse.bass as bass
import concourse.tile as tile
import concourse.mybir as mybir
from concourse.bass_utils import run_bass_kernel_spmd

dt = mybir.dt
F32 = dt.float32
F32R = dt.float32r
ALU = mybir.AluOpType
AF = mybir.ActivationFunctionType

B, C, H, W = 4, 128, 192, 192
HEADS = 8
C3 = 3 * C
HH = H // 2          # rows per core
SLAB = HH + 2        # input rows incl halo
Wp = W + 2           # padded width
RB = 16              # output rows per block
NB = HH // RB        # blocks per core
BPX = RB * Wp        # padded px per block (y tiles use W, x uses Wp)
NCHUNK = RB // 2     # 2-row chunks per block
CH_N = 2 * Wp        # matmul free size per chunk (388)
NPX = HH * W         # unpadded px per core (18432)
P4N = 2 * W          # phase-4 chunk px (384)
NP4 = NPX // P4N     # 48


def _split_multi_waits(nc, max_waits=1):
    # This walrus build rejects instructions carrying more than one sem
    # wait (and Drain carrying any); spill extras onto same-engine NoOps.
    ctr = 0
    for f in nc.m.functions:
        for bb in f.blocks:
            new = []
            changed = False
            for inst in bb.instructions:
                si = inst.sync_info
                nw = len(si.on_wait) if si is not None else 0
                limit = 0 if inst.opcode == "Drain" else max_waits
                if si is not None and nw > limit:
                    waits = list(si.on_wait)
                    keep = waits[nw - limit:] if limit else []
                    spill = waits[:nw - limit] if limit else waits
                    for w in spill:
                        ctr += 1
                        nop = mybir.InstNoOp(name=f"WSPLIT-{ctr}", ins=[], outs=[])
                        nop.engine = inst.engine
                        nop.sync_info = mybir.SyncInfo(on_wait=[w], on_update=[])
                        new.append(nop)
                    inst.sync_info = mybir.SyncInfo(
                        on_wait=keep, on_update=list(si.on_update))
                    changed = True
                new.append(inst)
            if changed:
                bb.instructions = new


_CACHE = {}


def _build():
    if "nc" in _CACHE:
        return _CACHE["nc"]
    nc = bass.Bass("TRN2", target_bir_lowering=False, debug=False, num_devices=8)

    # ---- I/O ----
    x_r = nc.dram_tensor("x_r", [C, SLAB, Wp], F32R, kind="ExternalInput")
    x_i = nc.dram_tensor("x_i", [C, SLAB, Wp], F32R, kind="ExternalInput")
    qkvT_r = nc.dram_tensor("qkvT_r", [C, C3], F32, kind="ExternalInput")
    qkvT_i = nc.dram_tensor("qkvT_i", [C, C3], F32, kind="ExternalInput")
    dwb_r = nc.dram_tensor("dwb_r", [C, 9, C3], F32, kind="ExternalInput")
    dwb_i = nc.dram_tensor("dwb_i", [C, 9, C3], F32, kind="ExternalInput")
    projT_r = nc.dram_tensor("projT_r", [C, C], F32R, kind="ExternalInput")
    projT_i = nc.dram_tensor("projT_i", [C, C], F32R, kind="ExternalInput")
    tempv_r = nc.dram_tensor("tempv_r", [C, 1], F32, kind="ExternalInput")
    tempv_i = nc.dram_tensor("tempv_i", [C, 1], F32, kind="ExternalInput")
    out_r = nc.dram_tensor("out_r", [C, HH, W], F32, kind="ExternalOutput")
    out_i = nc.dram_tensor("out_i", [C, HH, W], F32, kind="ExternalOutput")

    v_sr = nc.dram_tensor("v_sr", [C, NPX], F32R)
    v_si = nc.dram_tensor("v_si", [C, NPX], F32R)
    cc_in = nc.dram_tensor("cc_in", [C, 516], F32)
    cc_out = nc.dram_tensor("cc_out", [C, 516], F32)

    ident_d = nc.inline_tensor(np.eye(C, dtype=np.float32), name="ident")
    ones_d = nc.inline_tensor(np.ones((1, C), dtype=np.float32), name="ones1")
    _mask = np.zeros((C, C), np.float32)
    for h in range(HEADS):
        _mask[16 * h:16 * h + 16, 16 * h:16 * h + 16] = 1.0
    off_d = nc.inline_tensor((1.0 - _mask) * -1e30, name="blkoff")

    xr_flat = x_r.ap().rearrange("p r c -> p (r c)")
    xi_flat = x_i.ap().rearrange("p r c -> p (r c)")
    or_flat = out_r.ap().rearrange("p r c -> p (r c)")
    oi_flat = out_i.ap().rearrange("p r c -> p (r c)")

    with tile.TileContext(nc) as tc:
        with (
            tc.tile_pool(name="persist", bufs=1) as pp,
            tc.tile_pool(name="gram_ps", bufs=1, space="PSUM") as psg,
        ):
            # persistent tiles
            Mr = pp.tile([C, 9, C3], F32R)
            Mi = pp.tile([C, 9, C3], F32R)
            Ms = pp.tile([C, 9, C3], F32R)
            ident_t = pp.tile([C, C], F32)
            ones_t = pp.tile([1, C], F32)
            off_t = pp.tile([C, C], F32)
            pTr = pp.tile([C, C], F32R)
            pTi = pp.tile([C, C], F32R)
            pTin = pp.tile([C, C], F32R)
            tvr = pp.tile([C, 1], F32)
            tvi = pp.tile([C, 1], F32)
            AT = pp.tile([C, 3 * C], F32R)
            ssq_acc = pp.tile([C, 4, NB], F32)
            gram = psg.tile([C, 512], F32)

            nc.sync.dma_start(ident_t[:], ident_d.ap())
            nc.sync.dma_start(ones_t[:], ones_d.ap())
            nc.sync.dma_start(off_t[:], off_d.ap())
            nc.sync.dma_start(pTr[:], projT_r.ap())
            nc.sync.dma_start(pTi[:], projT_i.ap())
            nc.sync.dma_start(tvr[:], tempv_r.ap())
            nc.sync.dma_start(tvi[:], tempv_i.ap())
            nc.vector.tensor_scalar_mul(pTin[:], pTi[:], -1.0)

            # ---- phase 1: tap-weight combine ----
            with tc.tile_pool(name="prep", bufs=1) as prp:
                qr_t = prp.tile([C, C3], F32)
                qi_t = prp.tile([C, C3], F32)
                dr_t = prp.tile([C, 9, C3], F32)
                di_t = prp.tile([C, 9, C3], F32)
                s1 = prp.tile([C, C3], F32)
                s2 = prp.tile([C, C3], F32)
                nc.sync.dma_start(qr_t[:], qkvT_r.ap())
                nc.sync.dma_start(qi_t[:], qkvT_i.ap())
                nc.sync.dma_start(dr_t[:], dwb_r.ap())
                nc.sync.dma_start(di_t[:], dwb_i.ap())
                for t in range(9):
                    nc.vector.tensor_mul(s1[:], qr_t[:], dr_t[:, t, :])
                    nc.vector.tensor_mul(s2[:], qi_t[:], di_t[:, t, :])
                    nc.vector.tensor_sub(Mr[:, t, :], s1[:], s2[:])
                    nc.vector.tensor_mul(s1[:], qr_t[:], di_t[:, t, :])
                    nc.vector.tensor_mul(s2[:], qi_t[:], dr_t[:, t, :])
                    nc.vector.tensor_add(Mi[:, t, :], s1[:], s2[:])
                    nc.vector.tensor_add(Ms[:, t, :], Mr[:, t, :], Mi[:, t, :])

            # ---- phase 2: fused conv + Gram, streamed over blocks ----
            with (
                tc.tile_pool(name="xp", bufs=1) as xp,
                tc.tile_pool(name="yp", bufs=1) as yp,
                tc.tile_pool(name="qkp", bufs=2) as qkp,
                tc.tile_pool(name="sqp", bufs=1) as sqp,
                tc.tile_pool(name="sep", bufs=2) as sep,
                tc.tile_pool(name="m_ps", bufs=2, space="PSUM") as psm,
                tc.tile_pool(name="t_ps", bufs=1, space="PSUM") as pst,
            ):
                GN = 18 * Wp  # slab px per block
                first_gram = [True]
                for i in range(NB):
                    xr_t = xp.tile([C, GN + 2], F32R, tag="xr")
                    xi_t = xp.tile([C, GN + 2], F32R, tag="xi")
                    xs_t = xp.tile([C, GN + 2], F32R, tag="xs")
                    base = i * RB * Wp
                    nc.sync.dma_start(
                        xr_t[:, 1:GN + 1], xr_flat[:, base:base + GN])
                    nc.sync.dma_start(
                        xi_t[:, 1:GN + 1], xi_flat[:, base:base + GN])
                    nc.vector.tensor_add(
                        xs_t[:, 1:GN + 1], xr_t[:, 1:GN + 1], xi_t[:, 1:GN + 1])

                    q_r = yp.tile([C, RB, W], F32, tag="q_r")
                    q_i = yp.tile([C, RB, W], F32, tag="q_i")
                    k_r = yp.tile([C, RB, W], F32, tag="k_r")
                    k_i = yp.tile([C, RB, W], F32, tag="k_i")
                    v_r = yp.tile([C, RB, W], F32R, tag="v_r")
                    v_i = yp.tile([C, RB, W], F32R, tag="v_i")
                    ys = [(q_r, q_i), (k_r, k_i), (v_r, v_i)]

                    for tidx in range(3):
                        yr_t, yi_t = ys[tidx]
                        for j in range(NCHUNK):
                            m1 = psm.tile([C, CH_N], F32, tag="m1")
                            m2 = psm.tile([C, CH_N], F32, tag="m2")
                            m3 = psm.tile([C, CH_N], F32, tag="m3")
                            cb = 1 + (2 * j + 1) * Wp
                            for t in range(9):
                                off = cb + (t // 3 - 1) * Wp + (t % 3 - 1)
                                st, sp = (t == 0), (t == 8)
                                lsl = slice(tidx * C, tidx * C + C)
                                nc.tensor.matmul(
                                    m1[:], Mr[:, t, lsl],
                                    xr_t[:, off:off + CH_N], start=st, stop=sp)
                                nc.tensor.matmul(
                                    m2[:], Mi[:, t, lsl],
                                    xi_t[:, off:off + CH_N], start=st, stop=sp)
                                nc.tensor.matmul(
                                    m3[:], Ms[:, t, lsl],
                                    xs_t[:, off:off + CH_N], start=st, stop=sp)
                            c1 = sep.tile([C, CH_N], F32, tag="c1")
                            nc.vector.tensor_copy(c1[:], m1[:])
                            s12 = sep.tile([C, CH_N], F32, tag="s12")
                            nc.vector.tensor_add(s12[:], c1[:], m2[:])
                            c1v = c1[:].rearrange("p (r c) -> p r c", r=2)
                            m2v = m2[:].rearrange("p (r c) -> p r c", r=2)
                            m3v = m3[:].rearrange("p (r c) -> p r c", r=2)
                            s12v = s12[:].rearrange("p (r c) -> p r c", r=2)
                            rsl = slice(2 * j, 2 * j + 2)
                            nc.vector.tensor_sub(yr_t[:, rsl, :], c1v[:, :, 1:W + 1],
                                m2v[:, :, 1:W + 1])
                            nc.vector.tensor_sub(yi_t[:, rsl, :], m3v[:, :, 1:W + 1],
                                s12v[:, :, 1:W + 1])

                    # v out to scratch
                    vb = i * RB * W
                    nc.sync.dma_start(
                        v_sr.ap()[:, vb:vb + RB * W],
                        v_r[:].rearrange("p r c -> p (r c)"))
                    nc.sync.dma_start(
                        v_si.ap()[:, vb:vb + RB * W],
                        v_i[:].rearrange("p r c -> p (r c)"))

                    # transposes + Gram accumulation
                    flats = [q_r[:].rearrange("p r c -> p (r c)"),
                             q_i[:].rearrange("p r c -> p (r c)"),
                             k_r[:].rearrange("p r c -> p (r c)"),
                             k_i[:].rearrange("p r c -> p (r c)")]
                    nch = RB * W // C  # 24 transpose chunks
                    for cix in range(nch):
                        tp = pst.tile([C, 512], F32, tag="tp")
                        for k4 in range(4):
                            nc.tensor.transpose(
                                tp[:, k4 * C:(k4 + 1) * C],
                                flats[k4][:, cix * C:(cix + 1) * C], ident_t[:])
                        qk = qkp.tile([C, 512], F32R, tag="qk")
                        nc.vector.tensor_copy(qk[:], tp[:])
                        st = first_gram[0]
                        sp = (i == NB - 1) and (cix == nch - 1)
                        nc.tensor.matmul(
                            gram[:, 0:256], qk[:, 0:C], qk[:, 2 * C:4 * C],
                            start=st, stop=sp, skip_group_check=True)
                        nc.tensor.matmul(
                            gram[:, 256:512], qk[:, C:2 * C], qk[:, 2 * C:4 * C],
                            start=False, stop=sp, skip_group_check=True)
                        first_gram[0] = False

                    # sumsq via ACT square + accum
                    sq_t = sqp.tile([C, RB * W], F32, tag="sq")
                    for k4 in range(4):
                        nc.scalar.activation(
                            sq_t[:], flats[k4][:], AF.Square,
                            accum_out=ssq_acc[:, k4, i:i + 1])

            # ---- phase 3: allreduce + softmax ----
            with (
                tc.tile_pool(name="p3", bufs=1) as p3,
                tc.tile_pool(name="ps3", bufs=1, space="PSUM") as ps3,
            ):
                stage = p3.tile([C, 516], F32)
                nc.vector.tensor_copy(stage[:, 0:512], gram[:])
                nc.vector.tensor_reduce(
                    stage[:, 512:516], ssq_acc[:], axis=mybir.AxisListType.X,
                    op=ALU.add)
                nc.sync.dma_start(cc_in.ap(), stage[:])
                nc.gpsimd.collective_compute(
                    "AllReduce", ALU.add,
                    replica_groups=[[0, 1], [2, 3], [4, 5], [6, 7]],
                    ins=[cc_in.ap()], outs=[cc_out.ap()])
                P = p3.tile([C, 516], F32)
                nc.sync.dma_start(P[:], cc_out.ap())

                nrm = p3.tile([C, 4], F32)
                nc.scalar.activation(nrm[:], P[:, 512:516], AF.Sqrt)
                rsq = p3.tile([C, 4], F32)
                nc.vector.reciprocal(rsq[:], nrm[:])

                prow = ps3.tile([1, 256], F32)
                nc.tensor.transpose(prow[0:1, 0:C], rsq[:, 2:3], ident_t[:])
                nc.tensor.transpose(prow[0:1, C:2 * C], rsq[:, 3:4], ident_t[:])
                rowb = p3.tile([1, 256], F32)
                nc.vector.tensor_copy(rowb[:], prow[:])
                pbc = ps3.tile([C, 256], F32)
                nc.tensor.matmul(pbc[:], ones_t[:], rowb[:], start=True, stop=True)
                bc = p3.tile([C, 256], F32)
                nc.vector.tensor_copy(bc[:], pbc[:])

                S1s = p3.tile([C, 256], F32)
                S2s = p3.tile([C, 256], F32)
                nc.vector.scalar_tensor_tensor(
                    S1s[:], P[:, 0:256], rsq[:, 0:1], bc[:],
                    op0=ALU.mult, op1=ALU.mult)
                nc.vector.scalar_tensor_tensor(
                    S2s[:], P[:, 256:512], rsq[:, 1:2], bc[:],
                    op0=ALU.mult, op1=ALU.mult)
                ar = p3.tile([C, C], F32)
                ai = p3.tile([C, C], F32)
                nc.vector.tensor_sub(ar[:], S1s[:, 0:C], S2s[:, C:2 * C])
                nc.vector.tensor_add(ai[:], S1s[:, C:2 * C], S2s[:, 0:C])

                pA = ps3.tile([C, 3 * C], F32)
                for nidx, (logit, tv) in enumerate([(ar, tvr), (ai, tvi)]):
                    lg = p3.tile([C, C], F32, tag="lg")
                    nc.vector.scalar_tensor_tensor(
                        lg[:], logit[:], tv[:], off_t[:],
                        op0=ALU.mult, op1=ALU.add)
                    mx = p3.tile([C, 1], F32, tag="mx")
                    nc.vector.tensor_reduce(
                        mx[:], lg[:], axis=mybir.AxisListType.X, op=ALU.max)
                    nc.vector.tensor_scalar_sub(lg[:], lg[:], mx[:])
                    ex = p3.tile([C, C], F32, tag="ex")
                    nc.scalar.activation(ex[:], lg[:], AF.Exp)
                    sm = p3.tile([C, 1], F32, tag="sm")
                    nc.vector.tensor_reduce(
                        sm[:], ex[:], axis=mybir.AxisListType.X, op=ALU.add)
                    smi = p3.tile([C, 1], F32, tag="smi")
                    nc.vector.reciprocal(smi[:], sm[:])
                    Amat = p3.tile([C, C], F32, tag="Amat")
                    nc.vector.tensor_scalar_mul(Amat[:], ex[:], smi[:])
                    if nidx == 0:
                        nc.tensor.transpose(pA[:, 0:C], Amat[:], ident_t[:])
                    else:
                        nc.tensor.transpose(pA[:, C:2 * C], Amat[:], ident_t[:])
                        Ain = p3.tile([C, C], F32)
                        nc.vector.tensor_scalar_mul(Ain[:], Amat[:], -1.0)
                        nc.tensor.transpose(pA[:, 2 * C:3 * C], Ain[:], ident_t[:])
                nc.vector.tensor_copy(AT[:], pA[:])

            # ---- phase 4: attn@v + proj ----
            with (
                tc.tile_pool(name="vp", bufs=3) as vp,
                tc.tile_pool(name="op", bufs=2) as op_,
                tc.tile_pool(name="ps4", bufs=2, space="PSUM") as ps4,
                tc.tile_pool(name="ps4b", bufs=1, space="PSUM") as ps4b,
            ):
                for k in range(NP4):
                    pb = k * P4N
                    vr_c = vp.tile([C, P4N], F32R, tag="vr")
                    vi_c = vp.tile([C, P4N], F32R, tag="vi")
                    nc.sync.dma_start(vr_c[:], v_sr.ap()[:, pb:pb + P4N])
                    nc.sync.dma_start(vi_c[:], v_si.ap()[:, pb:pb + P4N])
                    por = ps4.tile([C, P4N], F32, tag="por")
                    poi = ps4.tile([C, P4N], F32, tag="poi")
                    nc.tensor.matmul(por[:], AT[:, 0:C], vr_c[:],
                                     start=True, stop=False)
                    nc.tensor.matmul(por[:], AT[:, 2 * C:3 * C], vi_c[:],
                                     start=False, stop=True)
                    nc.tensor.matmul(poi[:], AT[:, 0:C], vi_c[:],
                                     start=True, stop=False)
                    nc.tensor.matmul(poi[:], AT[:, C:2 * C], vr_c[:],
                                     start=False, stop=True)
                    ors = op_.tile([C, P4N], F32R, tag="ors")
                    ois = op_.tile([C, P4N], F32R, tag="ois")
                    nc.vector.tensor_copy(ors[:], por[:])
                    nc.vector.tensor_copy(ois[:], poi[:])
                    pfr = ps4b.tile([C, P4N], F32, tag="pfr")
                    pfi = ps4b.tile([C, P4N], F32, tag="pfi")
                    nc.tensor.matmul(pfr[:], pTr[:], ors[:],
                                     start=True, stop=False)
                    nc.tensor.matmul(pfr[:], pTin[:], ois[:],
                                     start=False, stop=True)
                    nc.tensor.matmul(pfi[:], pTr[:], ois[:],
                                     start=True, stop=False)
                    nc.tensor.matmul(pfi[:], pTi[:], ors[:],
                                     start=False, stop=True)
                    fr = op_.tile([C, P4N], F32, tag="fr")
                    fi = op_.tile([C, P4N], F32, tag="fi")
                    nc.vector.tensor_copy(fr[:], pfr[:])
                    nc.vector.tensor_copy(fi[:], pfi[:])
                    nc.sync.dma_start(or_flat[:, pb:pb + P4N], fr[:])
                    nc.sync.dma_start(oi_flat[:, pb:pb + P4N], fi[:])

    _split_multi_waits(nc)
    _CACHE["nc"] = nc
    return nc


def _host_inputs(x_real, x_imag, qkv_wr, qkv_wi, dw_wr, dw_wi,
                 proj_wr, proj_wi, temp_r, temp_i):
    f = np.float32
    qkvT_r = np.ascontiguousarray(np.asarray(qkv_wr, f).T)
    qkvT_i = np.ascontiguousarray(np.asarray(qkv_wi, f).T)
    dwt_r = np.asarray(dw_wr, f).reshape(C3, 9).T          # [9, 384]
    dwt_i = np.asarray(dw_wi, f).reshape(C3, 9).T
    dwb_r = np.ascontiguousarray(
        np.broadcast_to(dwt_r[None, :, :], (C, 9, C3)))
    dwb_i = np.ascontiguousarray(
        np.broadcast_to(dwt_i[None, :, :], (C, 9, C3)))
    projT_r = np.ascontiguousarray(np.asarray(proj_wr, f).T)
    projT_i = np.ascontiguousarray(np.asarray(proj_wi, f).T)
    tvr = np.repeat(np.asarray(temp_r, f).reshape(HEADS), 16).reshape(C, 1)
    tvi = np.repeat(np.asarray(temp_i, f).reshape(HEADS), 16).reshape(C, 1)
    tvr = np.ascontiguousarray(tvr)
    tvi = np.ascontiguousarray(tvi)

    xr = np.asarray(x_real, f)
    xi = np.asarray(x_imag, f)
    in_maps = []
    for core in range(8):
        b, hh = core // 2, core % 2
        lo = hh * HH - 1
        sl_r = np.zeros((C, SLAB, Wp), f)
        sl_i = np.zeros((C, SLAB, Wp), f)
        s0 = max(lo, 0)
        s1 = min(lo + SLAB, H)
        d0 = s0 - lo
        sl_r[:, d0:d0 + (s1 - s0), 1:W + 1] = xr[b, :, s0:s1, :]
        sl_i[:, d0:d0 + (s1 - s0), 1:W + 1] = xi[b, :, s0:s1, :]
        in_maps.append({
            "x_r": sl_r, "x_i": sl_i,
            "qkvT_r": qkvT_r, "qkvT_i": qkvT_i,
            "dwb_r": dwb_r, "dwb_i": dwb_i,
            "projT_r": projT_r, "projT_i": projT_i,
            "tempv_r": tvr, "tempv_i": tvi,
        })
    return in_maps


def kernel(**inputs):
    nc = _build()
    in_maps = _host_inputs(**inputs)
    res = run_bass_kernel_spmd(nc, in_maps, list(range(8)))
    out_r = np.empty((B, C, H, W), np.float32)
    out_i = np.empty((B, C, H, W), np.float32)
    for core in range(8):
        b, hh = core // 2, core % 2
        out_r[b, :, hh * HH:(hh + 1) * HH, :] = res.results[core]["out_r"]
        out_i[b, :, hh * HH:(hh + 1) * HH, :] = res.results[core]["out_i"]
    return out_r, out_i



# revision 14
# speedup vs baseline: 78.6127x; 78.6127x over previous
"""ComplexMDTA Trainium2 kernel.

Sharding: 8 cores = (batch 4) x (H halves 2). Each core computes its
(batch, 96-row) slice end-to-end. The only cross-core data dependencies
are the L2-norm sums and q@k^T Gram matrices (reductions over the full
H*W axis), handled with a tiny pairwise AllReduce between the two cores
sharing a batch.

Per core:
  phase 1: combine qkv 1x1-conv weights with depthwise 3x3 taps into 9
           fused complex tap matrices M[tap] (on device, DVE).
  phase 2: streamed over 16-row blocks: fused conv1x1+dwconv3x3 via 27
           PSUM-accumulated fp32r matmuls per 2-row chunk (Gauss 3-mult
           complex trick), epilogue spread over ACT/Pool/DVE into bf16
           q/k/v tiles, PE transposes (bf16) + Gram-matmul accumulation
           for q@kT, ACT-square sumsq. x_s = x_r + x_i is precomputed
           host-side and streamed as a third input.
  phase 3: pairwise AllReduce of Gram+sumsq partials, normalization,
           temperature, masked per-head softmax; the projection matrix
           is fused into the attention matrix on PE: FT = (A @ P)^T
           blocks, so phase 4 needs only one complex matmul per chunk.
  phase 4: streamed (P.A)@v from bf16 v scratch, DMA out fp32.
"""
import os
import sys

for _p in ('/opt/trn_rl_repo', '/root/.axon_site/_ro/trn_rl_repo'):
    if os.path.isdir(_p) and _p not in sys.path:
        sys.path.insert(0, _p)

import numpy as np
import concourse.bass as bass
import concourse.tile as tile
import concourse.mybir as mybir
from concourse.bass_utils import run_bass_kernel_spmd

dt = mybir.dt
F32 = dt.float32
F32R = dt.float32r
BF16 = dt.bfloat16
ALU = mybir.AluOpType
AF = mybir.ActivationFunctionType

B, C, H, W = 4, 128, 192, 192
HEADS = 8
C3 = 3 * C
HH = H // 2          # rows per core
SLAB = HH + 2        # input rows incl halo
Wp = W + 2           # padded width
RB = 16              # output rows per block
NB = HH // RB        # blocks per core
NCHUNK = RB // 2     # 2-row chunks per block
CH_N = 2 * Wp        # matmul free size per chunk (388)
NPX = HH * W         # unpadded px per core (18432)
P4N = 8 * W          # phase-4 chunk px (1536)
NP4 = NPX // P4N     # 12


def _split_multi_waits(nc, max_waits=1):
    # This walrus build rejects instructions carrying more than one sem
    # wait (and Drain carrying any); spill extras onto same-engine NoOps.
    ctr = 0
    for f in nc.m.functions:
        for bb in f.blocks:
            new = []
            changed = False
            for inst in bb.instructions:
                si = inst.sync_info
                nw = len(si.on_wait) if si is not None else 0
                limit = 0 if inst.opcode == "Drain" else max_waits
                if si is not None and nw > limit:
                    waits = list(si.on_wait)
                    keep = waits[nw - limit:] if limit else []
                    spill = waits[:nw - limit] if limit else waits
                    for w in spill:
                        ctr += 1
                        nop = mybir.InstNoOp(name=f"WSPLIT-{ctr}", ins=[], outs=[])
                        nop.engine = inst.engine
                        nop.sync_info = mybir.SyncInfo(on_wait=[w], on_update=[])
                        new.append(nop)
                    inst.sync_info = mybir.SyncInfo(
                        on_wait=keep, on_update=list(si.on_update))
                    changed = True
                new.append(inst)
            if changed:
                bb.instructions = new


_CACHE = {}


def _build():
    if "nc" in _CACHE:
        return _CACHE["nc"]
    nc = bass.Bass("TRN2", target_bir_lowering=False, debug=False, num_devices=8)

    # ---- I/O ----
    x_r = nc.dram_tensor("x_r", [C, SLAB, Wp], F32R, kind="ExternalInput")
    x_i = nc.dram_tensor("x_i", [C, SLAB, Wp], F32R, kind="ExternalInput")
    x_s = nc.dram_tensor("x_s", [C, SLAB, Wp], F32R, kind="ExternalInput")
    m_r = nc.dram_tensor("m_r", [C, 9, C3], F32R, kind="ExternalInput")
    m_i = nc.dram_tensor("m_i", [C, 9, C3], F32R, kind="ExternalInput")
    m_s = nc.dram_tensor("m_s", [C, 9, C3], F32R, kind="ExternalInput")
    projT_r = nc.dram_tensor("projT_r", [C, C], F32, kind="ExternalInput")
    projT_i = nc.dram_tensor("projT_i", [C, C], F32, kind="ExternalInput")
    tempv_r = nc.dram_tensor("tempv_r", [C, 1], F32, kind="ExternalInput")
    tempv_i = nc.dram_tensor("tempv_i", [C, 1], F32, kind="ExternalInput")
    out_r = nc.dram_tensor("out_r", [C, HH, W], F32, kind="ExternalOutput")
    out_i = nc.dram_tensor("out_i", [C, HH, W], F32, kind="ExternalOutput")

    v_sr = nc.dram_tensor("v_sr", [C, NPX], BF16)
    v_si = nc.dram_tensor("v_si", [C, NPX], BF16)
    cc_in = nc.dram_tensor("cc_in", [C, 516], F32)
    cc_out = nc.dram_tensor("cc_out", [C, 516], F32)

    ident_d = nc.inline_tensor(np.eye(C, dtype=np.float32), name="ident")
    import ml_dtypes
    identb_d = nc.inline_tensor(
        np.eye(C, dtype=ml_dtypes.bfloat16), name="identb")
    ones_d = nc.inline_tensor(np.ones((1, C), dtype=np.float32), name="ones1")
    _mask = np.zeros((C, C), np.float32)
    for h in range(HEADS):
        _mask[16 * h:16 * h + 16, 16 * h:16 * h + 16] = 1.0
    off_d = nc.inline_tensor((1.0 - _mask) * -1e30, name="blkoff")

    xr_flat = x_r.ap().rearrange("p r c -> p (r c)")
    xi_flat = x_i.ap().rearrange("p r c -> p (r c)")
    xs_flat = x_s.ap().rearrange("p r c -> p (r c)")
    or_flat = out_r.ap().rearrange("p r c -> p (r c)")
    oi_flat = out_i.ap().rearrange("p r c -> p (r c)")

    with tile.TileContext(nc) as tc:
        with (
            tc.tile_pool(name="persist", bufs=1) as pp,
            tc.tile_pool(name="gram_ps", bufs=1, space="PSUM") as psg,
        ):
            # persistent tiles
            Mr = pp.tile([C, 9, C3], F32R)
            Mi = pp.tile([C, 9, C3], F32R)
            Ms = pp.tile([C, 9, C3], F32R)
            ident_t = pp.tile([C, C], F32)
            identb_t = pp.tile([C, C], BF16)
            ones_t = pp.tile([1, C], F32)
            off_t = pp.tile([C, C], F32)
            pTr = pp.tile([C, C], F32)
            pTi = pp.tile([C, C], F32)
            pTin = pp.tile([C, C], F32)
            tvr = pp.tile([C, 1], F32)
            tvi = pp.tile([C, 1], F32)
            FT = pp.tile([C, 3 * C], BF16)
            ssq_acc = pp.tile([C, 4, NB], F32)
            gram = psg.tile([C, 512], F32)

            nc.sync.dma_start(ident_t[:], ident_d.ap())
            nc.sync.dma_start(identb_t[:], identb_d.ap())
            nc.sync.dma_start(ones_t[:], ones_d.ap())
            nc.sync.dma_start(off_t[:], off_d.ap())
            nc.sync.dma_start(pTr[:], projT_r.ap())
            nc.sync.dma_start(pTi[:], projT_i.ap())
            nc.sync.dma_start(tvr[:], tempv_r.ap())
            nc.sync.dma_start(tvi[:], tempv_i.ap())
            nc.sync.dma_start(Mr[:], m_r.ap())
            nc.sync.dma_start(Mi[:], m_i.ap())
            nc.sync.dma_start(Ms[:], m_s.ap())
            nc.vector.tensor_scalar_mul(pTin[:], pTi[:], -1.0)

            # ---- phase 2: fused conv + Gram, streamed over blocks ----
            with (
                tc.tile_pool(name="xp", bufs=2) as xp,
                tc.tile_pool(name="yp", bufs=1) as yp,
                tc.tile_pool(name="qkp", bufs=2) as qkp,
                tc.tile_pool(name="sqp", bufs=1) as sqp,
                tc.tile_pool(name="sep", bufs=2) as sep,
                tc.tile_pool(name="m_ps", bufs=2, space="PSUM") as psm,
                tc.tile_pool(name="t_ps", bufs=1, space="PSUM") as pst,
            ):
                GN = 18 * Wp  # slab px per block
                first_gram = [True]
                for i in range(NB):
                    xr_t = xp.tile([C, GN + 2], F32R, tag="xr")
                    xi_t = xp.tile([C, GN + 2], F32R, tag="xi")
                    xs_t = xp.tile([C, GN + 2], F32R, tag="xs")
                    base = i * RB * Wp
                    nc.sync.dma_start(
                        xr_t[:, 1:GN + 1], xr_flat[:, base:base + GN])
                    nc.sync.dma_start(
                        xi_t[:, 1:GN + 1], xi_flat[:, base:base + GN])
                    nc.sync.dma_start(
                        xs_t[:, 1:GN + 1], xs_flat[:, base:base + GN])

                    q_r = yp.tile([C, RB, W], BF16, tag="q_r")
                    q_i = yp.tile([C, RB, W], BF16, tag="q_i")
                    k_r = yp.tile([C, RB, W], BF16, tag="k_r")
                    k_i = yp.tile([C, RB, W], BF16, tag="k_i")
                    v_r = yp.tile([C, RB, W], BF16, tag="v_r")
                    v_i = yp.tile([C, RB, W], BF16, tag="v_i")
                    ys = [(q_r, q_i), (k_r, k_i), (v_r, v_i)]

                    for tidx in range(3):
                        yr_t, yi_t = ys[tidx]
                        for j in range(NCHUNK):
                            m1 = psm.tile([C, CH_N], F32, tag="m1")
                            m2 = psm.tile([C, CH_N], F32, tag="m2")
                            m3 = psm.tile([C, CH_N], F32, tag="m3")
                            cb = 1 + (2 * j + 1) * Wp
                            for t in range(9):
                                off = cb + (t // 3 - 1) * Wp + (t % 3 - 1)
                                st, sp = (t == 0), (t == 8)
                                lsl = slice(tidx * C, tidx * C + C)
                                nc.tensor.matmul(
                                    m1[:], Mr[:, t, lsl],
                                    xr_t[:, off:off + CH_N], start=st, stop=sp)
                                nc.tensor.matmul(
                                    m2[:], Mi[:, t, lsl],
                                    xi_t[:, off:off + CH_N], start=st, stop=sp)
                                nc.tensor.matmul(
                                    m3[:], Ms[:, t, lsl],
                                    xs_t[:, off:off + CH_N], start=st, stop=sp)
                            c1 = sep.tile([C, CH_N], F32, tag="c1")
                            nc.scalar.activation(c1[:], m1[:], AF.Copy)
                            s12 = sep.tile([C, CH_N], F32, tag="s12")
                            nc.vector.tensor_add(s12[:], c1[:], m2[:])
                            c1v = c1[:].rearrange("p (r c) -> p r c", r=2)
                            m2v = m2[:].rearrange("p (r c) -> p r c", r=2)
                            m3v = m3[:].rearrange("p (r c) -> p r c", r=2)
                            s12v = s12[:].rearrange("p (r c) -> p r c", r=2)
                            rsl = slice(2 * j, 2 * j + 2)
                            nc.vector.tensor_sub(yr_t[:, rsl, :], c1v[:, :, 1:W + 1],
                                m2v[:, :, 1:W + 1])
                            nc.vector.tensor_sub(yi_t[:, rsl, :], m3v[:, :, 1:W + 1],
                                s12v[:, :, 1:W + 1])

                    # v out to scratch (bf16)
                    vb = i * RB * W
                    nc.sync.dma_start(
                        v_sr.ap()[:, vb:vb + RB * W],
                        v_r[:].rearrange("p r c -> p (r c)"))
                    nc.sync.dma_start(
                        v_si.ap()[:, vb:vb + RB * W],
                        v_i[:].rearrange("p r c -> p (r c)"))

                    # transposes + Gram accumulation (bf16)
                    flats = [q_r[:].rearrange("p r c -> p (r c)"),
                             q_i[:].rearrange("p r c -> p (r c)"),
                             k_r[:].rearrange("p r c -> p (r c)"),
                             k_i[:].rearrange("p r c -> p (r c)")]
                    nch = RB * W // C  # 24 transpose chunks
                    for cix in range(nch):
                        tp = pst.tile([C, 512], BF16, tag="tp")
                        for k4 in range(4):
                            nc.tensor.transpose(
                                tp[:, k4 * C:(k4 + 1) * C],
                                flats[k4][:, cix * C:(cix + 1) * C], identb_t[:])
                        qk = qkp.tile([C, 512], BF16, tag="qk")
                        nc.scalar.activation(qk[:], tp[:], AF.Copy)
                        st = first_gram[0]
                        sp = (i == NB - 1) and (cix == nch - 1)
                        nc.tensor.matmul(
                            gram[:, 0:256], qk[:, 0:C], qk[:, 2 * C:4 * C],
                            start=st, stop=sp, skip_group_check=True)
                        nc.tensor.matmul(
                            gram[:, 256:512], qk[:, C:2 * C], qk[:, 2 * C:4 * C],
                            start=False, stop=sp, skip_group_check=True)
                        first_gram[0] = False

                    # sumsq via ACT square + accum
                    sq_t = sqp.tile([C, RB * W], F32, tag="sq")
                    for k4 in range(4):
                        nc.scalar.activation(
                            sq_t[:], flats[k4][:], AF.Square,
                            accum_out=ssq_acc[:, k4, i:i + 1])

            # ---- phase 3: allreduce + softmax + proj fusion ----
            with (
                tc.tile_pool(name="p3", bufs=1) as p3,
                tc.tile_pool(name="ps3", bufs=1, space="PSUM") as ps3,
            ):
                stage = p3.tile([C, 516], F32)
                nc.vector.tensor_copy(stage[:, 0:512], gram[:])
                nc.vector.tensor_reduce(
                    stage[:, 512:516], ssq_acc[:], axis=mybir.AxisListType.X,
                    op=ALU.add)
                nc.sync.dma_start(cc_in.ap(), stage[:])
                nc.gpsimd.collective_compute(
                    "AllReduce", ALU.add,
                    replica_groups=[[0, 1], [2, 3], [4, 5], [6, 7]],
                    ins=[cc_in.ap()], outs=[cc_out.ap()])
                P = p3.tile([C, 516], F32)
                nc.sync.dma_start(P[:], cc_out.ap())

                nrm = p3.tile([C, 4], F32)
                nc.scalar.activation(nrm[:], P[:, 512:516], AF.Sqrt)
                rsq = p3.tile([C, 4], F32)
                nc.vector.reciprocal(rsq[:], nrm[:])

                prow = ps3.tile([1, 256], F32)
                nc.tensor.transpose(prow[0:1, 0:C], rsq[:, 2:3], ident_t[:])
                nc.tensor.transpose(prow[0:1, C:2 * C], rsq[:, 3:4], ident_t[:])
                rowb = p3.tile([1, 256], F32)
                nc.vector.tensor_copy(rowb[:], prow[:])
                pbc = ps3.tile([C, 256], F32)
                nc.tensor.matmul(pbc[:], ones_t[:], rowb[:], start=True, stop=True)
                bc = p3.tile([C, 256], F32)
                nc.vector.tensor_copy(bc[:], pbc[:])

                S1s = p3.tile([C, 256], F32)
                S2s = p3.tile([C, 256], F32)
                nc.vector.scalar_tensor_tensor(
                    S1s[:], P[:, 0:256], rsq[:, 0:1], bc[:],
                    op0=ALU.mult, op1=ALU.mult)
                nc.vector.scalar_tensor_tensor(
                    S2s[:], P[:, 256:512], rsq[:, 1:2], bc[:],
                    op0=ALU.mult, op1=ALU.mult)
                ar = p3.tile([C, C], F32)
                ai = p3.tile([C, C], F32)
                nc.vector.tensor_sub(ar[:], S1s[:, 0:C], S2s[:, C:2 * C])
                nc.vector.tensor_add(ai[:], S1s[:, C:2 * C], S2s[:, 0:C])

                Amats = []
                for nidx, (logit, tv) in enumerate([(ar, tvr), (ai, tvi)]):
                    lg = p3.tile([C, C], F32, tag=f"lg{nidx}")
                    nc.vector.scalar_tensor_tensor(
                        lg[:], logit[:], tv[:], off_t[:],
                        op0=ALU.mult, op1=ALU.add)
                    mx = p3.tile([C, 1], F32, tag=f"mx{nidx}")
                    nc.vector.tensor_reduce(
                        mx[:], lg[:], axis=mybir.AxisListType.X, op=ALU.max)
                    nc.vector.tensor_scalar_sub(lg[:], lg[:], mx[:])
                    ex = p3.tile([C, C], F32, tag=f"ex{nidx}")
                    nc.scalar.activation(ex[:], lg[:], AF.Exp)
                    sm = p3.tile([C, 1], F32, tag=f"sm{nidx}")
                    nc.vector.tensor_reduce(
                        sm[:], ex[:], axis=mybir.AxisListType.X, op=ALU.add)
                    smi = p3.tile([C, 1], F32, tag=f"smi{nidx}")
                    nc.vector.reciprocal(smi[:], sm[:])
                    Amat = p3.tile([C, C], F32, tag=f"Amat{nidx}")
                    nc.vector.tensor_scalar_mul(Amat[:], ex[:], smi[:])
                    Amats.append(Amat)

                # fold proj into attention: FT = [(PA)r^T | (PA)i^T | -(PA)i^T]
                # (PA)^T = A^T P^T ; matmul(out, lhsT=Amat, rhs=pT) = A^T @ pT
                Ar_t, Ai_t = Amats
                ftr_ps = ps3.tile([C, C], F32, tag="ftr")
                fti_ps = ps3.tile([C, C], F32, tag="fti")
                nc.tensor.matmul(ftr_ps[:], Ar_t[:], pTr[:],
                                 start=True, stop=False)
                nc.tensor.matmul(ftr_ps[:], Ai_t[:], pTin[:],
                                 start=False, stop=True)
                nc.tensor.matmul(fti_ps[:], Ar_t[:], pTi[:],
                                 start=True, stop=False)
                nc.tensor.matmul(fti_ps[:], Ai_t[:], pTr[:],
                                 start=False, stop=True)
                nc.vector.tensor_copy(FT[:, 0:C], ftr_ps[:])
                nc.scalar.activation(FT[:, C:2 * C], fti_ps[:], AF.Copy)
                nc.vector.tensor_scalar_mul(FT[:, 2 * C:3 * C], fti_ps[:], -1.0)

            # ---- phase 4: fused (P.A)@v, big chunks, 512-wide matmuls ----
            with (
                tc.tile_pool(name="vp", bufs=2) as vp,
                tc.tile_pool(name="op", bufs=2) as op_,
                tc.tile_pool(name="ps4", bufs=2, space="PSUM") as ps4,
            ):
                for k in range(NP4):
                    pb = k * P4N
                    vr_c = vp.tile([C, P4N], BF16, tag="vr")
                    vi_c = vp.tile([C, P4N], BF16, tag="vi")
                    nc.sync.dma_start(vr_c[:], v_sr.ap()[:, pb:pb + P4N])
                    nc.sync.dma_start(vi_c[:], v_si.ap()[:, pb:pb + P4N])
                    fr = op_.tile([C, P4N], F32, tag="fr")
                    fi = op_.tile([C, P4N], F32, tag="fi")
                    for s in range(3):
                        sl = slice(s * 512, (s + 1) * 512)
                        pfr = ps4.tile([C, 512], F32, tag="pfr")
                        pfi = ps4.tile([C, 512], F32, tag="pfi")
                        nc.tensor.matmul(pfr[:], FT[:, 0:C], vr_c[:, sl],
                                         start=True, stop=False)
                        nc.tensor.matmul(pfr[:], FT[:, 2 * C:3 * C], vi_c[:, sl],
                                         start=False, stop=True)
                        nc.tensor.matmul(pfi[:], FT[:, 0:C], vi_c[:, sl],
                                         start=True, stop=False)
                        nc.tensor.matmul(pfi[:], FT[:, C:2 * C], vr_c[:, sl],
                                         start=False, stop=True)
                        nc.scalar.activation(fr[:, sl], pfr[:], AF.Copy)
                        nc.vector.tensor_copy(fi[:, sl], pfi[:])
                    nc.sync.dma_start(or_flat[:, pb:pb + P4N], fr[:])
                    nc.sync.dma_start(oi_flat[:, pb:pb + P4N], fi[:])

    _split_multi_waits(nc)
    _CACHE["nc"] = nc
    return nc


def _host_inputs(x_real, x_imag, qkv_wr, qkv_wi, dw_wr, dw_wi,
                 proj_wr, proj_wi, temp_r, temp_i):
    f = np.float32
    qkvT_r = np.ascontiguousarray(np.asarray(qkv_wr, f).T)
    qkvT_i = np.ascontiguousarray(np.asarray(qkv_wi, f).T)
    dwt_r = np.asarray(dw_wr, f).reshape(C3, 9).T          # [9, 384]
    dwt_i = np.asarray(dw_wi, f).reshape(C3, 9).T
    # fused complex tap matrices, [in-ch, tap, out-ch]
    m_r = qkvT_r[:, None, :] * dwt_r[None] - qkvT_i[:, None, :] * dwt_i[None]
    m_i = qkvT_r[:, None, :] * dwt_i[None] + qkvT_i[:, None, :] * dwt_r[None]
    m_r = np.ascontiguousarray(m_r, f)
    m_i = np.ascontiguousarray(m_i, f)
    m_s = m_r + m_i
    projT_r = np.ascontiguousarray(np.asarray(proj_wr, f).T)
    projT_i = np.ascontiguousarray(np.asarray(proj_wi, f).T)
    tvr = np.repeat(np.asarray(temp_r, f).reshape(HEADS), 16).reshape(C, 1)
    tvi = np.repeat(np.asarray(temp_i, f).reshape(HEADS), 16).reshape(C, 1)
    tvr = np.ascontiguousarray(tvr)
    tvi = np.ascontiguousarray(tvi)

    xr = np.asarray(x_real, f)
    xi = np.asarray(x_imag, f)
    in_maps = []
    for core in range(8):
        b, hh = core // 2, core % 2
        lo = hh * HH - 1
        sl_r = np.zeros((C, SLAB, Wp), f)
        sl_i = np.zeros((C, SLAB, Wp), f)
        s0 = max(lo, 0)
        s1 = min(lo + SLAB, H)
        d0 = s0 - lo
        sl_r[:, d0:d0 + (s1 - s0), 1:W + 1] = xr[b, :, s0:s1, :]
        sl_i[:, d0:d0 + (s1 - s0), 1:W + 1] = xi[b, :, s0:s1, :]
        in_maps.append({
            "x_r": sl_r, "x_i": sl_i, "x_s": sl_r + sl_i,
            "m_r": m_r, "m_i": m_i, "m_s": m_s,
            "projT_r": projT_r, "projT_i": projT_i,
            "tempv_r": tvr, "tempv_i": tvi,
        })
    return in_maps


def kernel(**inputs):
    nc = _build()
    in_maps = _host_inputs(**inputs)
    res = run_bass_kernel_spmd(nc, in_maps, list(range(8)))
    out_r = np.empty((B, C, H, W), np.float32)
    out_i = np.empty((B, C, H, W), np.float32)
    for core in range(8):
        b, hh = core // 2, core % 2
        out_r[b, :, hh * HH:(hh + 1) * HH, :] = res.results[core]["out_r"]
        out_i[b, :, hh * HH:(hh + 1) * HH, :] = res.results[core]["out_i"]
    return out_r, out_i


# revision 18
# speedup vs baseline: 98.2072x; 1.2493x over previous
"""ComplexMDTA Trainium2 kernel.

Sharding: 8 cores = (batch 4) x (H halves 2). Each core computes its
(batch, 96-row) slice end-to-end. The only cross-core data dependencies
are the L2-norm sums and q@k^T Gram matrices (reductions over the full
H*W axis), handled with a tiny pairwise AllReduce between the two cores
sharing a batch.

Per core:
  phase 1: combine qkv 1x1-conv weights with depthwise 3x3 taps into 9
           fused complex tap matrices M[tap] (on device, DVE).
  phase 2: streamed over 16-row blocks: fused conv1x1+dwconv3x3 via 27
           PSUM-accumulated fp32r matmuls per 2-row chunk (Gauss 3-mult
           complex trick), epilogue spread over ACT/Pool/DVE into bf16
           q/k/v tiles, PE transposes (bf16) + Gram-matmul accumulation
           for q@kT, ACT-square sumsq. x_s = x_r + x_i is precomputed
           host-side and streamed as a third input.
  phase 3: pairwise AllReduce of Gram+sumsq partials, normalization,
           temperature, masked per-head softmax; the projection matrix
           is fused into the attention matrix on PE: FT = (A @ P)^T
           blocks, so phase 4 needs only one complex matmul per chunk.
  phase 4: streamed (P.A)@v from bf16 v scratch, DMA out fp32.
"""
import os
import sys

for _p in ('/opt/trn_rl_repo', '/root/.axon_site/_ro/trn_rl_repo'):
    if os.path.isdir(_p) and _p not in sys.path:
        sys.path.insert(0, _p)

import numpy as np
import concourse.bass as bass
import concourse.tile as tile
import concourse.mybir as mybir
from concourse.bass_utils import run_bass_kernel_spmd

dt = mybir.dt
F32 = dt.float32
F32R = dt.float32r
BF16 = dt.bfloat16
ALU = mybir.AluOpType
AF = mybir.ActivationFunctionType

B, C, H, W = 4, 128, 192, 192
HEADS = 8
C3 = 3 * C
HH = H // 2          # rows per core
SLAB = HH + 2        # input rows incl halo
Wp = W + 2           # padded width
RB = 16              # output rows per block
NB = HH // RB        # blocks per core
NCHUNK = RB // 2     # 2-row chunks per block
CH_N = 2 * Wp        # matmul free size per chunk (388)
NPX = HH * W         # unpadded px per core (18432)
P4N = 8 * W          # phase-4 chunk px (1536)
NP4 = NPX // P4N     # 12


def _split_multi_waits(nc, max_waits=1):
    # This walrus build rejects instructions carrying more than one sem
    # wait (and Drain carrying any); spill extras onto same-engine NoOps.
    ctr = 0
    for f in nc.m.functions:
        for bb in f.blocks:
            new = []
            changed = False
            for inst in bb.instructions:
                si = inst.sync_info
                nw = len(si.on_wait) if si is not None else 0
                limit = 0 if inst.opcode == "Drain" else max_waits
                if si is not None and nw > limit:
                    waits = list(si.on_wait)
                    keep = waits[nw - limit:] if limit else []
                    spill = waits[:nw - limit] if limit else waits
                    for w in spill:
                        ctr += 1
                        nop = mybir.InstNoOp(name=f"WSPLIT-{ctr}", ins=[], outs=[])
                        nop.engine = inst.engine
                        nop.sync_info = mybir.SyncInfo(on_wait=[w], on_update=[])
                        new.append(nop)
                    inst.sync_info = mybir.SyncInfo(
                        on_wait=keep, on_update=list(si.on_update))
                    changed = True
                new.append(inst)
            if changed:
                bb.instructions = new


_CACHE = {}


def _build():
    if "nc" in _CACHE:
        return _CACHE["nc"]
    nc = bass.Bass("TRN2", target_bir_lowering=False, debug=False, num_devices=8)

    # ---- I/O ----
    x_r = nc.dram_tensor("x_r", [C, SLAB, Wp], F32R, kind="ExternalInput")
    x_i = nc.dram_tensor("x_i", [C, SLAB, Wp], F32R, kind="ExternalInput")
    x_s = nc.dram_tensor("x_s", [C, SLAB, Wp], F32R, kind="ExternalInput")
    m_r = nc.dram_tensor("m_r", [C, 9, C3], F32R, kind="ExternalInput")
    m_i = nc.dram_tensor("m_i", [C, 9, C3], F32R, kind="ExternalInput")
    m_s = nc.dram_tensor("m_s", [C, 9, C3], F32R, kind="ExternalInput")
    projT_r = nc.dram_tensor("projT_r", [C, C], F32, kind="ExternalInput")
    projT_i = nc.dram_tensor("projT_i", [C, C], F32, kind="ExternalInput")
    tempv_r = nc.dram_tensor("tempv_r", [C, 1], F32, kind="ExternalInput")
    tempv_i = nc.dram_tensor("tempv_i", [C, 1], F32, kind="ExternalInput")
    out_r = nc.dram_tensor("out_r", [C, HH, W], F32, kind="ExternalOutput")
    out_i = nc.dram_tensor("out_i", [C, HH, W], F32, kind="ExternalOutput")

    v_sr = nc.dram_tensor("v_sr", [C, NPX], BF16)
    v_si = nc.dram_tensor("v_si", [C, NPX], BF16)
    cc_in = nc.dram_tensor("cc_in", [C, 516], F32)
    cc_out = nc.dram_tensor("cc_out", [C, 516], F32)

    ident_d = nc.inline_tensor(np.eye(C, dtype=np.float32), name="ident")
    import ml_dtypes
    identb_d = nc.inline_tensor(
        np.eye(C, dtype=ml_dtypes.bfloat16), name="identb")
    ones_d = nc.inline_tensor(np.ones((1, C), dtype=np.float32), name="ones1")
    _mask = np.zeros((C, C), np.float32)
    for h in range(HEADS):
        _mask[16 * h:16 * h + 16, 16 * h:16 * h + 16] = 1.0
    off_d = nc.inline_tensor((1.0 - _mask) * -1e30, name="blkoff")

    xr_flat = x_r.ap().rearrange("p r c -> p (r c)")
    xi_flat = x_i.ap().rearrange("p r c -> p (r c)")
    xs_flat = x_s.ap().rearrange("p r c -> p (r c)")
    or_flat = out_r.ap().rearrange("p r c -> p (r c)")
    oi_flat = out_i.ap().rearrange("p r c -> p (r c)")

    with tile.TileContext(nc) as tc:
        with (
            tc.tile_pool(name="persist", bufs=1) as pp,
            tc.tile_pool(name="gram_ps", bufs=1, space="PSUM") as psg,
        ):
            # persistent tiles
            Mr = pp.tile([C, 9, C3], F32R)
            Mi = pp.tile([C, 9, C3], F32R)
            Ms = pp.tile([C, 9, C3], F32R)
            ident_t = pp.tile([C, C], F32)
            identb_t = pp.tile([C, C], BF16)
            ones_t = pp.tile([1, C], F32)
            off_t = pp.tile([C, C], F32)
            pTr = pp.tile([C, C], F32)
            pTi = pp.tile([C, C], F32)
            pTin = pp.tile([C, C], F32)
            tvr = pp.tile([C, 1], F32)
            tvi = pp.tile([C, 1], F32)
            FT = pp.tile([C, 3 * C], BF16)
            ssq_acc = pp.tile([C, 4, NB], F32)
            gram = psg.tile([C, 512], F32)

            nc.sync.dma_start(ident_t[:], ident_d.ap())
            nc.sync.dma_start(identb_t[:], identb_d.ap())
            nc.sync.dma_start(ones_t[:], ones_d.ap())
            nc.sync.dma_start(off_t[:], off_d.ap())
            nc.sync.dma_start(pTr[:], projT_r.ap())
            nc.sync.dma_start(pTi[:], projT_i.ap())
            nc.sync.dma_start(tvr[:], tempv_r.ap())
            nc.sync.dma_start(tvi[:], tempv_i.ap())
            nc.sync.dma_start(Mr[:], m_r.ap())
            nc.sync.dma_start(Mi[:], m_i.ap())
            nc.sync.dma_start(Ms[:], m_s.ap())
            nc.vector.tensor_scalar_mul(pTin[:], pTi[:], -1.0)

            # ---- phase 2: fused conv + Gram, streamed over blocks ----
            with (
                tc.tile_pool(name="xp", bufs=2) as xp,
                tc.tile_pool(name="yp", bufs=1) as yp,
                tc.tile_pool(name="qkp", bufs=2) as qkp,
                tc.tile_pool(name="sqp", bufs=1) as sqp,
                tc.tile_pool(name="sep", bufs=2) as sep,
                tc.tile_pool(name="m_ps", bufs=2, space="PSUM") as psm,
                tc.tile_pool(name="t_ps", bufs=1, space="PSUM") as pst,
            ):
                GN = 18 * Wp  # slab px per block
                first_gram = [True]
                for i in range(NB):
                    xr_t = xp.tile([C, GN + 2], F32R, tag="xr")
                    xi_t = xp.tile([C, GN + 2], F32R, tag="xi")
                    xs_t = xp.tile([C, GN + 2], F32R, tag="xs")
                    base = i * RB * Wp
                    nc.sync.dma_start(
                        xr_t[:, 1:GN + 1], xr_flat[:, base:base + GN])
                    nc.sync.dma_start(
                        xi_t[:, 1:GN + 1], xi_flat[:, base:base + GN])
                    nc.sync.dma_start(
                        xs_t[:, 1:GN + 1], xs_flat[:, base:base + GN])

                    q_r = yp.tile([C, RB, W], BF16, tag="q_r")
                    q_i = yp.tile([C, RB, W], BF16, tag="q_i")
                    k_r = yp.tile([C, RB, W], BF16, tag="k_r")
                    k_i = yp.tile([C, RB, W], BF16, tag="k_i")
                    v_r = yp.tile([C, RB, W], BF16, tag="v_r")
                    v_i = yp.tile([C, RB, W], BF16, tag="v_i")
                    ys = [(q_r, q_i), (k_r, k_i), (v_r, v_i)]

                    for tidx in range(3):
                        yr_t, yi_t = ys[tidx]
                        for j in range(NCHUNK):
                            m1 = psm.tile([C, CH_N], F32, tag="m1")
                            m2 = psm.tile([C, CH_N], F32, tag="m2")
                            m3 = psm.tile([C, CH_N], F32, tag="m3")
                            cb = 1 + (2 * j + 1) * Wp
                            for t in range(9):
                                off = cb + (t // 3 - 1) * Wp + (t % 3 - 1)
                                st, sp = (t == 0), (t == 8)
                                lsl = slice(tidx * C, tidx * C + C)
                                nc.tensor.matmul(
                                    m1[:], Mr[:, t, lsl],
                                    xr_t[:, off:off + CH_N], start=st, stop=sp)
                                nc.tensor.matmul(
                                    m2[:], Mi[:, t, lsl],
                                    xi_t[:, off:off + CH_N], start=st, stop=sp)
                                nc.tensor.matmul(
                                    m3[:], Ms[:, t, lsl],
                                    xs_t[:, off:off + CH_N], start=st, stop=sp)
                            c1 = sep.tile([C, CH_N], F32, tag="c1")
                            nc.scalar.activation(c1[:], m1[:], AF.Copy)
                            s12 = sep.tile([C, CH_N], F32, tag="s12")
                            nc.vector.tensor_add(s12[:], c1[:], m2[:])
                            c1v = c1[:].rearrange("p (r c) -> p r c", r=2)
                            m2v = m2[:].rearrange("p (r c) -> p r c", r=2)
                            m3v = m3[:].rearrange("p (r c) -> p r c", r=2)
                            s12v = s12[:].rearrange("p (r c) -> p r c", r=2)
                            rsl = slice(2 * j, 2 * j + 2)
                            nc.vector.tensor_sub(yr_t[:, rsl, :], c1v[:, :, 1:W + 1],
                                m2v[:, :, 1:W + 1])
                            nc.vector.tensor_sub(yi_t[:, rsl, :], m3v[:, :, 1:W + 1],
                                s12v[:, :, 1:W + 1])

                    # v out to scratch (bf16)
                    vb = i * RB * W
                    nc.sync.dma_start(
                        v_sr.ap()[:, vb:vb + RB * W],
                        v_r[:].rearrange("p r c -> p (r c)"))
                    nc.sync.dma_start(
                        v_si.ap()[:, vb:vb + RB * W],
                        v_i[:].rearrange("p r c -> p (r c)"))

                    # transposes + Gram accumulation (bf16)
                    flats = [q_r[:].rearrange("p r c -> p (r c)"),
                             q_i[:].rearrange("p r c -> p (r c)"),
                             k_r[:].rearrange("p r c -> p (r c)"),
                             k_i[:].rearrange("p r c -> p (r c)")]
                    nch = RB * W // C  # 24 transpose chunks
                    for cix in range(nch):
                        tp = pst.tile([C, 512], BF16, tag="tp")
                        for k4 in range(4):
                            nc.tensor.transpose(
                                tp[:, k4 * C:(k4 + 1) * C],
                                flats[k4][:, cix * C:(cix + 1) * C], identb_t[:])
                        qk = qkp.tile([C, 512], BF16, tag="qk")
                        nc.scalar.activation(qk[:], tp[:], AF.Copy)
                        st = first_gram[0]
                        sp = (i == NB - 1) and (cix == nch - 1)
                        nc.tensor.matmul(
                            gram[:, 0:256], qk[:, 0:C], qk[:, 2 * C:4 * C],
                            start=st, stop=sp, skip_group_check=True)
                        nc.tensor.matmul(
                            gram[:, 256:512], qk[:, C:2 * C], qk[:, 2 * C:4 * C],
                            start=False, stop=sp, skip_group_check=True)
                        first_gram[0] = False

                    # sumsq via ACT square + accum
                    sq_t = sqp.tile([C, RB * W], BF16, tag="sq")
                    for k4 in range(4):
                        nc.scalar.activation(
                            sq_t[:], flats[k4][:], AF.Square,
                            accum_out=ssq_acc[:, k4, i:i + 1])

            # ---- phase 3: allreduce + softmax + proj fusion ----
            with (
                tc.tile_pool(name="p3", bufs=1) as p3,
                tc.tile_pool(name="ps3", bufs=1, space="PSUM") as ps3,
            ):
                stage = p3.tile([C, 516], F32)
                nc.vector.tensor_copy(stage[:, 0:512], gram[:])
                nc.vector.tensor_reduce(
                    stage[:, 512:516], ssq_acc[:], axis=mybir.AxisListType.X,
                    op=ALU.add)
                nc.sync.dma_start(cc_in.ap(), stage[:])
                nc.gpsimd.collective_compute(
                    "AllReduce", ALU.add,
                    replica_groups=[[0, 1], [2, 3], [4, 5], [6, 7]],
                    ins=[cc_in.ap()], outs=[cc_out.ap()])
                P = p3.tile([C, 516], F32)
                nc.sync.dma_start(P[:], cc_out.ap())

                nrm = p3.tile([C, 4], F32)
                nc.scalar.activation(nrm[:], P[:, 512:516], AF.Sqrt)
                rsq = p3.tile([C, 4], F32)
                nc.vector.reciprocal(rsq[:], nrm[:])

                prow = ps3.tile([1, 256], F32)
                nc.tensor.transpose(prow[0:1, 0:C], rsq[:, 2:3], ident_t[:])
                nc.tensor.transpose(prow[0:1, C:2 * C], rsq[:, 3:4], ident_t[:])
                rowb = p3.tile([1, 256], F32)
                nc.vector.tensor_copy(rowb[:], prow[:])
                pbc = ps3.tile([C, 256], F32)
                nc.tensor.matmul(pbc[:], ones_t[:], rowb[:], start=True, stop=True)
                bc = p3.tile([C, 256], F32)
                nc.vector.tensor_copy(bc[:], pbc[:])

                S1s = p3.tile([C, 256], F32)
                S2s = p3.tile([C, 256], F32)
                nc.vector.scalar_tensor_tensor(
                    S1s[:], P[:, 0:256], rsq[:, 0:1], bc[:],
                    op0=ALU.mult, op1=ALU.mult)
                nc.vector.scalar_tensor_tensor(
                    S2s[:], P[:, 256:512], rsq[:, 1:2], bc[:],
                    op0=ALU.mult, op1=ALU.mult)
                ar = p3.tile([C, C], F32)
                ai = p3.tile([C, C], F32)
                nc.vector.tensor_sub(ar[:], S1s[:, 0:C], S2s[:, C:2 * C])
                nc.vector.tensor_add(ai[:], S1s[:, C:2 * C], S2s[:, 0:C])

                Amats = []
                for nidx, (logit, tv) in enumerate([(ar, tvr), (ai, tvi)]):
                    lg = p3.tile([C, C], F32, tag=f"lg{nidx}")
                    nc.vector.scalar_tensor_tensor(
                        lg[:], logit[:], tv[:], off_t[:],
                        op0=ALU.mult, op1=ALU.add)
                    mx = p3.tile([C, 1], F32, tag=f"mx{nidx}")
                    nc.vector.tensor_reduce(
                        mx[:], lg[:], axis=mybir.AxisListType.X, op=ALU.max)
                    nc.vector.tensor_scalar_sub(lg[:], lg[:], mx[:])
                    ex = p3.tile([C, C], F32, tag=f"ex{nidx}")
                    nc.scalar.activation(ex[:], lg[:], AF.Exp)
                    sm = p3.tile([C, 1], F32, tag=f"sm{nidx}")
                    nc.vector.tensor_reduce(
                        sm[:], ex[:], axis=mybir.AxisListType.X, op=ALU.add)
                    smi = p3.tile([C, 1], F32, tag=f"smi{nidx}")
                    nc.vector.reciprocal(smi[:], sm[:])
                    Amat = p3.tile([C, C], F32, tag=f"Amat{nidx}")
                    nc.vector.tensor_scalar_mul(Amat[:], ex[:], smi[:])
                    Amats.append(Amat)

                # fold proj into attention: FT = [(PA)r^T | (PA)i^T | -(PA)i^T]
                # (PA)^T = A^T P^T ; matmul(out, lhsT=Amat, rhs=pT) = A^T @ pT
                Ar_t, Ai_t = Amats
                ftr_ps = ps3.tile([C, C], F32, tag="ftr")
                fti_ps = ps3.tile([C, C], F32, tag="fti")
                nc.tensor.matmul(ftr_ps[:], Ar_t[:], pTr[:],
                                 start=True, stop=False)
                nc.tensor.matmul(ftr_ps[:], Ai_t[:], pTin[:],
                                 start=False, stop=True)
                nc.tensor.matmul(fti_ps[:], Ar_t[:], pTi[:],
                                 start=True, stop=False)
                nc.tensor.matmul(fti_ps[:], Ai_t[:], pTr[:],
                                 start=False, stop=True)
                nc.vector.tensor_copy(FT[:, 0:C], ftr_ps[:])
                nc.scalar.activation(FT[:, C:2 * C], fti_ps[:], AF.Copy)
                nc.vector.tensor_scalar_mul(FT[:, 2 * C:3 * C], fti_ps[:], -1.0)

            # ---- phase 4: fused (P.A)@v, big chunks, 512-wide matmuls ----
            with (
                tc.tile_pool(name="vp", bufs=12) as vp,
                tc.tile_pool(name="op", bufs=2) as op_,
                tc.tile_pool(name="ps4", bufs=2, space="PSUM") as ps4,
            ):
                for k in range(NP4):
                    pb = k * P4N
                    vr_c = vp.tile([C, P4N], BF16, tag="vr")
                    vi_c = vp.tile([C, P4N], BF16, tag="vi")
                    nc.sync.dma_start(vr_c[:], v_sr.ap()[:, pb:pb + P4N])
                    nc.sync.dma_start(vi_c[:], v_si.ap()[:, pb:pb + P4N])
                    fr = op_.tile([C, P4N], F32, tag="fr")
                    fi = op_.tile([C, P4N], F32, tag="fi")
                    for s in range(3):
                        sl = slice(s * 512, (s + 1) * 512)
                        pfr = ps4.tile([C, 512], F32, tag="pfr")
                        pfi = ps4.tile([C, 512], F32, tag="pfi")
                        nc.tensor.matmul(pfr[:], FT[:, 0:C], vr_c[:, sl],
                                         start=True, stop=False)
                        nc.tensor.matmul(pfr[:], FT[:, 2 * C:3 * C], vi_c[:, sl],
                                         start=False, stop=True)
                        nc.tensor.matmul(pfi[:], FT[:, 0:C], vi_c[:, sl],
                                         start=True, stop=False)
                        nc.tensor.matmul(pfi[:], FT[:, C:2 * C], vr_c[:, sl],
                                         start=False, stop=True)
                        nc.scalar.activation(fr[:, sl], pfr[:], AF.Copy)
                        nc.vector.tensor_copy(fi[:, sl], pfi[:])
                    nc.sync.dma_start(or_flat[:, pb:pb + P4N], fr[:])
                    nc.sync.dma_start(oi_flat[:, pb:pb + P4N], fi[:])

    _split_multi_waits(nc)
    _CACHE["nc"] = nc
    return nc


def _host_inputs(x_real, x_imag, qkv_wr, qkv_wi, dw_wr, dw_wi,
                 proj_wr, proj_wi, temp_r, temp_i):
    f = np.float32
    qkvT_r = np.ascontiguousarray(np.asarray(qkv_wr, f).T)
    qkvT_i = np.ascontiguousarray(np.asarray(qkv_wi, f).T)
    dwt_r = np.asarray(dw_wr, f).reshape(C3, 9).T          # [9, 384]
    dwt_i = np.asarray(dw_wi, f).reshape(C3, 9).T
    # fused complex tap matrices, [in-ch, tap, out-ch]
    m_r = qkvT_r[:, None, :] * dwt_r[None] - qkvT_i[:, None, :] * dwt_i[None]
    m_i = qkvT_r[:, None, :] * dwt_i[None] + qkvT_i[:, None, :] * dwt_r[None]
    m_r = np.ascontiguousarray(m_r, f)
    m_i = np.ascontiguousarray(m_i, f)
    m_s = m_r + m_i
    projT_r = np.ascontiguousarray(np.asarray(proj_wr, f).T)
    projT_i = np.ascontiguousarray(np.asarray(proj_wi, f).T)
    tvr = np.repeat(np.asarray(temp_r, f).reshape(HEADS), 16).reshape(C, 1)
    tvi = np.repeat(np.asarray(temp_i, f).reshape(HEADS), 16).reshape(C, 1)
    tvr = np.ascontiguousarray(tvr)
    tvi = np.ascontiguousarray(tvi)

    xr = np.asarray(x_real, f)
    xi = np.asarray(x_imag, f)
    in_maps = []
    for core in range(8):
        b, hh = core // 2, core % 2
        lo = hh * HH - 1
        sl_r = np.zeros((C, SLAB, Wp), f)
        sl_i = np.zeros((C, SLAB, Wp), f)
        s0 = max(lo, 0)
        s1 = min(lo + SLAB, H)
        d0 = s0 - lo
        sl_r[:, d0:d0 + (s1 - s0), 1:W + 1] = xr[b, :, s0:s1, :]
        sl_i[:, d0:d0 + (s1 - s0), 1:W + 1] = xi[b, :, s0:s1, :]
        in_maps.append({
            "x_r": sl_r, "x_i": sl_i, "x_s": sl_r + sl_i,
            "m_r": m_r, "m_i": m_i, "m_s": m_s,
            "projT_r": projT_r, "projT_i": projT_i,
            "tempv_r": tvr, "tempv_i": tvi,
        })
    return in_maps


def kernel(**inputs):
    nc = _build()
    in_maps = _host_inputs(**inputs)
    res = run_bass_kernel_spmd(nc, in_maps, list(range(8)))
    out_r = np.empty((B, C, H, W), np.float32)
    out_i = np.empty((B, C, H, W), np.float32)
    for core in range(8):
        b, hh = core // 2, core % 2
        out_r[b, :, hh * HH:(hh + 1) * HH, :] = res.results[core]["out_r"]
        out_i[b, :, hh * HH:(hh + 1) * HH, :] = res.results[core]["out_i"]
    return out_r, out_i
